# revision 74
# baseline (speedup 1.0000x reference)
"""Trainium2 Bass kernel for nn_AttentionModel_PCA (sparse_attention).

loss = pseudo-likelihood of a Potts-style attention model + regularizer.

M-sharded data-parallel across 8 NeuronCores (Q/K/V replicated; scalar
partials summed on host). Per core (ML=256 of M=2048):

P1: eT_h = K_h^T Q_h on the PE in f32r; unnormalized exp(e-20) -> bf16 on
    ACT; per-(h,i) column sums via 1/256-ones matmuls; reciprocal row;
    PE-broadcast of the reciprocal; DVE fused normalize+quantize to
    sf8 = fp8_e4m3(256*softmax), layout [j_p, (jb, h, i)] so the jb pair
    feeds DoubleRow matmuls. One-hot masks O_s[j,(jb,m)] in fp8 via DVE
    compares (exact 0/1).
P2: Y'(h,s)[i,m] = sf8_h @ O_s as fp8 DoubleRow matmuls (j=256 contracted
    per instruction), 672 matmuls total; PSUM [128,1536] s-triples
    evacuated with scale 2^-4 to fp8 staging (rotating ACT/DVE/Pool), one
    DMA per h into ydram[hs, (p, ib, m)] with 512B-contiguous runs. The
    DMA performs the (i,m)->partition flatten (hs to partitions).
P3: me[im, q] = sum_hs Yt[hs, im] Vcat[hs, q] with Yt slabs read back
    [128 hs-part, im], DoubleRow over the 256-row chunk pair + an 80-row
    fp8 matmul, accumulating [128, 16*21] PSUM groups; exp (ACT, scale
    2^-4) -> Pool segment-reduce -> lsum; weighted selected-state energy
    via one fused DVE tensor_tensor_reduce against a host-precomputed
    w*onehot(Z) mask; single Ln pass at the end (2 act-table loads total).
Gram regularizer: 256 fp8 DoubleRow matmuls of sf8 columns, scaled 2^-16.

Outputs per core: [sum_m w*(sel-lge), reg']; host combines.
"""
import numpy as np
import ml_dtypes
from contextlib import ExitStack

import concourse.bass as bass
import concourse.tile as tile
from concourse import bacc, mybir

F32 = mybir.dt.float32
F32R = mybir.dt.float32r
BF16 = mybir.dt.bfloat16
I32 = mybir.dt.int32
F8 = mybir.dt.float8e4

H, D, N, S = 16, 64, 256, 21
ML = 256
HS = H * S            # 336
NB = 512              # im blocks of 128 per core (N*ML/128)
NSUP = 8              # superblocks
SUPW = 8192           # im columns per superblock
GSUP = SUPW // (16 * 128)  # me-groups per superblock
LAMBD = 0.001
N_CORES = 8
EXP_SHIFT = 20.0
SFSC = 128.0          # sf8 = 128*softmax (fp8e4m3 max is 240, so not 256)
YSC = 2.0 ** -3       # evac scale: psum holds 128*Y, store 16*Y in fp8
MESC = 2.0 ** -4      # me_psum holds 16*me
DR = mybir.MatmulPerfMode.DoubleRow
AF = mybir.ActivationFunctionType
ALU = mybir.AluOpType
AXX = mybir.AxisListType.X


def _build_kernel(n_cores=N_CORES, reps=1, dbg=False):
    nc = bacc.Bacc("TRN2", target_bir_lowering=False, debug=False,
                   num_devices=n_cores)

    k_in = nc.dram_tensor("k_in", [D, H * N], F32R, kind="ExternalInput")
    q_in = nc.dram_tensor("q_in", [D, H * N], F32R, kind="ExternalInput")
    z_in = nc.dram_tensor("z_in", [2, 128, ML], I32, kind="ExternalInput")
    wsel_in = nc.dram_tensor("wsel_in", [128, NB * S], F8, kind="ExternalInput")
    wexp_in = nc.dram_tensor("wexp_in", [128, NB], F32, kind="ExternalInput")
    vdr_in = nc.dram_tensor("vdr_in", [128, 2 * S], F8, kind="ExternalInput")
    v2_in = nc.dram_tensor("v2_in", [80, S], F8, kind="ExternalInput")
    w2_in = nc.dram_tensor("w2_in", [16, 16], F32, kind="ExternalInput")
    out_partial = nc.dram_tensor("partial", [1, 2], F32, kind="ExternalOutput")
    # split by hs-chunk so phase-3 reads only wait on the writes they need
    ydA = nc.dram_tensor("ydA", [128, 128 * 2 * ML], F8, kind="Internal")
    ydB = nc.dram_tensor("ydB", [128, 128 * 2 * ML], F8, kind="Internal")
    ydC = nc.dram_tensor("ydC", [80, 128 * 2 * ML], F8, kind="Internal")
    dbg_t = {}
    if dbg:
        dbg_t["sft8"] = nc.dram_tensor("dbg_sft8", [128, 2 * H * N], F8,
                                       kind="ExternalOutput")
        dbg_t["yd"] = nc.dram_tensor("dbg_yd", [128, 2048], F8,
                                     kind="ExternalOutput")
        dbg_t["lsum"] = nc.dram_tensor("dbg_lsum", [128, NB], F32,
                                       kind="ExternalOutput")
        dbg_t["selacc"] = nc.dram_tensor("dbg_selacc", [128, 33], F32,
                                         kind="ExternalOutput")
        dbg_t["yt01"] = nc.dram_tensor("dbg_yt01", [128, 2 * SUPW], F8,
                                       kind="ExternalOutput")
        dbg_t["yt2"] = nc.dram_tensor("dbg_yt2", [80, SUPW], F8,
                                      kind="ExternalOutput")
        dbg_t["me0"] = nc.dram_tensor("dbg_me0", [128, 16 * S], F32,
                                      kind="ExternalOutput")

    with ExitStack() as ctx:
        tc = ctx.enter_context(tile.TileContext(nc))
        pers = ctx.enter_context(tc.tile_pool(name="pers", bufs=1))
        p3y = ctx.enter_context(tc.tile_pool(name="p3y", bufs=4))
        ones_inv = pers.tile([128, 1], BF16)
        nc.vector.memset(ones_inv[:], 1.0 / SFSC)
        ones1w = pers.tile([1, 128], BF16)
        nc.vector.memset(ones1w[:], 1.0)
        ones128 = pers.tile([128, 1], F32)
        nc.vector.memset(ones128[:], 1.0)
        negshift = pers.tile([128, 1], F32)
        nc.vector.memset(negshift[:], -EXP_SHIFT)
        sft8 = pers.tile([128, 2 * H * N], F8, name="sft8")
        o8 = pers.tile([128, S * 2 * ML], F8, name="o8")
        reg_sb = pers.tile([1, 2], F32)
        w2_sb = pers.tile([16, 16], F32)
        vdr_sb = pers.tile([128, 2 * S], F8)
        v2_sb = pers.tile([80, S], F8)
        wsel_sb = pers.tile([128, NB * S], F8, name="wsel")
        wexp_sb = pers.tile([128, NB], F32)

        def _late_inputs():
            # non-P1-critical inputs: issue after P1's k/q/z DMAs
            nc.sync.dma_start(vdr_sb[:], vdr_in[:, :])
            nc.sync.dma_start(v2_sb[:], v2_in[:, :])
            nc.sync.dma_start(w2_sb[:], w2_in[:, :])
            nc.sync.dma_start(wsel_sb[:], wsel_in[:, :])
            nc.sync.dma_start(wexp_sb[:], wexp_in[:, :])

        yd = (ydA, ydB, ydC)
        for _rep in range(reps):
            _phase1(nc, tc, k_in, q_in, z_in, ones_inv, ones1w, negshift,
                    sft8, o8, _late_inputs)
            _phase2(nc, tc, sft8, o8, yd)
            _gram(nc, tc, sft8, w2_sb, ones128, reg_sb)
            if dbg:
                nc.sync.dma_start(dbg_t["sft8"][:, :], sft8[:])
                nc.sync.dma_start(dbg_t["yd"][:, :], ydA[0:128, 0:2048])
            _phase3(nc, tc, yd, vdr_sb, v2_sb, wsel_sb, wexp_sb,
                    ones128, reg_sb, out_partial, p3y, dbg_t)

    nc.compile()
    return nc


def _phase1(nc, tc, k_in, q_in, z_in, ones_inv, ones1w, negshift, sft8, o8,
            late_inputs=None):
    sft8v = sft8[:].rearrange("p (c h i) -> p c h i", c=2, h=H)
    with tc.tile_pool(name="p1", bufs=1) as p1, \
         tc.tile_pool(name="p1et", bufs=4, space="PSUM") as p1et, \
         tc.tile_pool(name="p1cs", bufs=2, space="PSUM") as p1cs, \
         tc.tile_pool(name="p1cr", bufs=2, space="PSUM") as p1cr:
        k_sb = p1.tile([D, H * N], F32R)
        nc.sync.dma_start(k_sb[:], k_in[:, :])
        q_sb = p1.tile([D, H * N], F32R)
        nc.sync.dma_start(q_sb[:], q_in[:, :])

        # one-hot masks in fp8 (exact 0/1), layout [j_p, (jb, m)] per s
        zj = p1.tile([128, 2 * ML], I32)
        nc.sync.dma_start(zj[:].rearrange("p (c m) -> p c m", c=2),
                          z_in[:, :, :].rearrange("c p m -> p c m"))
        if late_inputs is not None:
            late_inputs()
        zjf = p1.tile([128, 2 * ML], F32)
        nc.gpsimd.tensor_copy(zjf[:], zj[:])
        for s in range(S):
            eng = nc.gpsimd if s % 2 == 0 else nc.vector
            eng.tensor_scalar(
                out=o8[:, s * 2 * ML:(s + 1) * 2 * ML], in0=zjf[:],
                scalar1=float(s), scalar2=None, op0=ALU.is_equal)

        sftu = p1.tile([128, 2 * H * N], BF16, name="sftu")
        sftuv = sftu[:].rearrange("p (c h i) -> p c h i", c=2, h=H)
        for h in range(H):
            cs_ps = p1cs.tile([1, N], F32, tag="cs")
            for jb in range(2):
                et_ps = p1et.tile([128, N], F32, tag="et")
                nc.tensor.matmul(
                    et_ps[:],
                    k_sb[:, h * N + jb * 128: h * N + jb * 128 + 128],
                    q_sb[:, h * N:(h + 1) * N],
                    start=True, stop=True)
                nc.scalar.activation(sftuv[:, jb, h, :], et_ps[:],
                                     AF.Exp, bias=negshift[:, :])
                nc.tensor.matmul(cs_ps[:], ones_inv[:],
                                 sftuv[:, jb, h, :],
                                 start=(jb == 0), stop=(jb == 1))
            crow = p1.tile([1, N], BF16, tag="crow")
            with nc.allow_low_precision("softmax reciprocal in bf16 is fine"):
                nc.vector.reciprocal(crow[:], cs_ps[:])
            crep_ps = p1cr.tile([128, N], F32, tag="crep")
            nc.tensor.matmul(crep_ps[:], ones1w[:], crow[:],
                             start=True, stop=True)
            for jb in range(2):
                nc.vector.tensor_tensor(
                    out=sft8v[:, jb, h, :], in0=sftuv[:, jb, h, :],
                    in1=crep_ps[:], op=ALU.mult)


def _phase2(nc, tc, sft8, o8, yd):
    sft8v = sft8[:].rearrange("p (c h i) -> p c h i", c=2, h=H)
    bases = ((0, 0, 128), (1, 128, 128), (2, 256, 80))
    with tc.tile_pool(name="p2sb", bufs=3) as p2sb, \
         tc.tile_pool(name="p2ps", bufs=2, space="PSUM") as p2ps:
        engines = [nc.scalar, nc.vector]
        ei = 0
        for h in range(H):
            stg = p2sb.tile([128, S * 512], F8, tag="stg", name="stg")
            for st in range(7):
                ps = p2ps.tile([128, 3 * 512], F32, tag="yps")
                for sl in range(3):
                    s = st * 3 + sl
                    for ib in range(2):
                        nc.tensor.matmul(
                            ps[:, sl * 512 + ib * ML: sl * 512 + ib * ML + ML],
                            sft8v[:, :, h, ib * 128:(ib + 1) * 128],
                            o8[:, s * 2 * ML:(s + 1) * 2 * ML]
                                .rearrange("p (c m) -> p c m", c=2),
                            start=True, stop=True, perf_mode=DR)
                eng = engines[0 if (ei % 9) in (0, 2, 4, 6, 8) else 1]
                ei += 1
                dst = stg[:, st * 1536:(st + 1) * 1536]
                if eng is nc.scalar:
                    nc.scalar.mul(dst, ps[:], YSC)
                else:
                    eng.tensor_scalar(out=dst, in0=ps[:], scalar1=YSC,
                                      scalar2=None, op0=ALU.mult)
            lo, hi = h * S, (h + 1) * S
            for t_idx, base, size in bases:
                a, b = max(lo, base), min(hi, base + size)
                if a >= b:
                    continue
                s0, s1 = a - lo, b - lo
                nc.sync.dma_start(
                    yd[t_idx][a - base:b - base, :]
                        .rearrange("s (p x) -> p s x", p=128),
                    stg[:, s0 * 512:s1 * 512]
                        .rearrange("p (s x) -> p s x", s=s1 - s0))


def _gram(nc, tc, sft8, w2_sb, ones128, reg_sb):
    sft8g = sft8[:].rearrange("p (c h i) -> p c h i", c=2, h=H)
    with tc.tile_pool(name="gr", bufs=1) as gr, \
         tc.tile_pool(name="grps", bufs=1, space="PSUM") as grps:
        mm_ps = grps.tile([16, 16], F32, tag="mm")
        for i in range(N):
            for jb in range(2):
                nc.tensor.matmul(mm_ps[:], sft8g[:, jb, :, i],
                                 sft8g[:, jb, :, i],
                                 start=(i == 0 and jb == 0),
                                 stop=(i == N - 1 and jb == 1))
        mw = gr.tile([16, 16], F32)
        nc.vector.tensor_tensor(out=mw[:], in0=mm_ps[:], in1=w2_sb[:],
                                op=ALU.mult)
        mwr = gr.tile([16, 1], F32)
        nc.vector.reduce_sum(mwr[:], mw[:], axis=AXX)
        reg_ps = grps.tile([1, 1], F32, tag="regp")
        nc.tensor.matmul(reg_ps[:], mwr[:], ones128[:16, :], start=True,
                         stop=True)
        nc.vector.tensor_copy(reg_sb[:, 1:2], reg_ps[:])


def _phase3(nc, tc, yd, vdr_sb, v2_sb, wsel_sb, wexp_sb, ones128,
            reg_sb, out_partial, p3y, dbg_t=None):
    vdrv = vdr_sb[:].rearrange("p (c q) -> p c q", c=2)
    with tc.tile_pool(name="p3", bufs=1) as p3, \
         tc.tile_pool(name="p3x", bufs=3) as p3x, \
         tc.tile_pool(name="p3ps", bufs=4, space="PSUM") as p3ps:
        lsum = p3.tile([128, NB], F32, name="lsum")
        esel = p3.tile([128, NB], F32, name="esel")
        for sup in range(NSUP):
            yt01 = p3y.tile([128, 2 * SUPW], F8, tag="yt01", name="yt01")
            yt01v_ = yt01[:].rearrange("p (c x) -> p c x", c=2)
            nc.gpsimd.dma_start(yt01v_[:, 0, :],
                                yd[0][:, sup * SUPW:(sup + 1) * SUPW])
            nc.gpsimd.dma_start(yt01v_[:, 1, :],
                                yd[1][:, sup * SUPW:(sup + 1) * SUPW])
            yt2 = p3y.tile([80, SUPW], F8, tag="yt2", name="yt2")
            nc.gpsimd.dma_start(yt2[:], yd[2][:, sup * SUPW:(sup + 1) * SUPW])
            yt01v = yt01[:].rearrange("p (c x) -> p c x", c=2)
            for g in range(GSUP):
                me = p3ps.tile([128, 16 * S], F32, tag="me")
                for blk in range(16):
                    off = (g * 16 + blk) * 128
                    nc.tensor.matmul(me[:, blk * S:(blk + 1) * S],
                                     yt01v[:, 0, off:off + 128], vdrv[:, 0, :],
                                     start=True, stop=False)
                    nc.tensor.matmul(me[:, blk * S:(blk + 1) * S],
                                     yt01v[:, 1, off:off + 128], vdrv[:, 1, :],
                                     start=False, stop=False)
                    nc.tensor.matmul(me[:, blk * S:(blk + 1) * S],
                                     yt2[:, off:off + 128], v2_sb[:, :],
                                     start=False, stop=True)
                gg = sup * GSUP + g
                if dbg_t and sup == 0 and g == 0:
                    me_dbg = p3x.tile([128, 16 * S], F32, tag="medbg")
                    nc.vector.tensor_copy(me_dbg[:], me[:])
                    nc.sync.dma_start(dbg_t["me0"][:, :], me_dbg[:])
                    nc.sync.dma_start(dbg_t["yt01"][:, :], yt01[:])
                    nc.sync.dma_start(dbg_t["yt2"][:, :], yt2[:])
                expo = p3x.tile([128, 16 * S], F32, tag="expo")
                nc.scalar.activation(expo[:], me[:], AF.Exp, scale=MESC)
                nc.vector.reduce_sum(
                    lsum[:, gg * 16:(gg + 1) * 16],
                    expo[:].rearrange("p (b q) -> p b q", q=S), axis=AXX)
                scr = p3x.tile([128, 16 * S], F32, tag="scr")
                nc.vector.tensor_tensor(
                    out=scr[:], in0=expo[:],
                    in1=wsel_sb[:, gg * 16 * S:(gg + 1) * 16 * S],
                    op=ALU.mult)
                nc.vector.reduce_sum(
                    esel[:, gg * 16:(gg + 1) * 16],
                    scr[:].rearrange("p (b q) -> p b q", q=S), axis=AXX)
        if dbg_t:
            nc.sync.dma_start(dbg_t["lsum"][:, :], lsum[:])
            nc.sync.dma_start(dbg_t["selacc"][:, 0:32], esel[:, 0:32])
        lge = p3.tile([128, NB], F32)
        nc.scalar.activation(lge[:], lsum[:], AF.Ln)
        lsel = p3.tile([128, NB], F32)
        nc.scalar.activation(lsel[:], esel[:], AF.Ln)
        dif = p3.tile([128, NB], F32)
        nc.vector.tensor_sub(dif[:], lsel[:], lge[:])
        scr2 = p3.tile([128, NB], F32)
        nc.vector.tensor_tensor(out=scr2[:], in0=dif[:], in1=wexp_sb[:],
                                op=ALU.mult)
        diff = p3.tile([128, 1], F32)
        nc.vector.reduce_sum(diff[:], scr2[:], axis=AXX)
        with tc.tile_pool(name="plps", bufs=1, space="PSUM") as plps:
            pl_ps = plps.tile([1, 1], F32, tag="pl")
            nc.tensor.matmul(pl_ps[:], diff[:], ones128[:], start=True,
                             stop=True)
            nc.vector.tensor_copy(reg_sb[:, 0:1], pl_ps[:])
        nc.sync.dma_start(out_partial[:, :], reg_sb[:])


# ===================== host side: shard, run, combine =====================

F8NP = ml_dtypes.float8_e4m3


def _prep_core_inputs(Z, weights, Q, K, V, core, n_cores=N_CORES):
    Z = np.asarray(Z)
    w = np.asarray(weights, np.float32)
    msl = slice(core * ML, (core + 1) * ML)
    zc = np.ascontiguousarray(Z[:, msl]).astype(np.int32)     # [j=256, m=256]
    wc = w[msl]                                               # [256]

    # block b = p*4 + ib*2 + mh ; partition ml ; i = ib*128+p ; m = mh*128+ml
    b = np.arange(NB)
    p_, ib_, mh_ = b // 4, (b // 2) % 2, b % 2
    i_of_b = ib_ * 128 + p_                                   # [NB]
    ml_ = np.arange(128)
    m_of = mh_[None, :] * 128 + ml_[:, None]                  # [128, NB]
    zsel = zc[i_of_b[None, :].repeat(128, 0), m_of]           # [128, NB]
    wm = wc[m_of]                                             # [128, NB]
    q_ar = np.arange(S)
    wsel = (zsel[:, :, None] == q_ar[None, None, :]).astype(np.float32)
    wsel = np.ascontiguousarray(wsel.reshape(128, NB * S)).astype(F8NP)

    vcat = np.transpose(np.asarray(V, np.float32), (0, 2, 1)).reshape(HS, S)
    v8 = vcat.astype(F8NP)
    vdr = np.empty((128, 2, S), F8NP)
    vdr[:, 0, :] = v8[0:128]
    vdr[:, 1, :] = v8[128:256]
    v2 = np.ascontiguousarray(v8[256:336])

    vv = np.asarray(V, np.float32).reshape(H, -1)
    w2 = (vv @ vv.T) / (SFSC * SFSC)

    return {
        "k_in": np.ascontiguousarray(
            np.asarray(K, np.float32).transpose(1, 0, 2).reshape(D, H * N)),
        "q_in": np.ascontiguousarray(
            np.asarray(Q, np.float32).transpose(1, 0, 2).reshape(D, H * N)),
        "z_in": np.ascontiguousarray(zc.reshape(2, 128, ML)),
        "wsel_in": wsel,
        "wexp_in": np.ascontiguousarray(wm.astype(np.float32)),
        "vdr_in": np.ascontiguousarray(vdr.reshape(128, 2 * S)),
        "v2_in": v2,
        "w2_in": np.ascontiguousarray(w2.astype(np.float32)),
    }


def _make_runner(nc, n_cores):
    """jit once; reuse. Based on bass2jax.run_bass_via_pjrt (axon path)."""
    import jax
    from jax.sharding import Mesh, PartitionSpec
    from jax.experimental.shard_map import shard_map
    from concourse import bass2jax

    bass2jax.install_neuronx_cc_hook()
    partition_name = (nc.partition_id_tensor.name
                      if nc.partition_id_tensor else None)
    in_names, out_names, out_avals, zero_outs = [], [], [], []
    for alloc in nc.m.functions[0].allocations:
        if not isinstance(alloc, mybir.MemoryLocationSet):
            continue
        name = alloc.memorylocations[0].name
        if alloc.kind == "ExternalInput":
            if name != partition_name:
                in_names.append(name)
        elif alloc.kind == "ExternalOutput":
            out_names.append(name)
            shape = tuple(alloc.tensor_shape)
            dtype = mybir.dt.np(alloc.dtype)
            out_avals.append(jax.core.ShapedArray(shape, dtype))
            zero_outs.append(np.zeros(shape, dtype))
    n_params = len(in_names)
    n_outs = len(out_names)
    all_in_names = in_names + out_names
    if partition_name is not None:
        all_in_names = all_in_names + [partition_name]

    def _body(*args):
        operands = list(args)
        if partition_name is not None:
            operands.append(bass2jax.partition_id_tensor())
        outs = bass2jax._bass_exec_p.bind(
            *operands,
            out_avals=tuple(out_avals),
            in_names=tuple(all_in_names),
            out_names=tuple(out_names),
            lowering_input_output_aliases=(),
            sim_require_finite=True,
            sim_require_nnan=True,
            nc=nc,
        )
        return tuple(outs)

    donate = tuple(range(n_params, n_params + n_outs))
    devices = jax.devices()[:n_cores]
    mesh = Mesh(np.asarray(devices), ("core",))
    in_specs = (PartitionSpec("core"),) * (n_params + n_outs)
    out_specs = (PartitionSpec("core"),) * n_outs
    jf = jax.jit(
        shard_map(_body, mesh=mesh, in_specs=in_specs, out_specs=out_specs,
                  check_rep=False),
        donate_argnums=donate, keep_unused=True,
    )

    def run(in_maps):
        concat_in = [
            np.concatenate([np.asarray(in_maps[c][n]) for c in range(n_cores)],
                           axis=0)
            for n in in_names
        ]
        concat_zeros = [
            np.zeros((n_cores * z.shape[0], *z.shape[1:]), z.dtype)
            for z in zero_outs
        ]
        outs = jf(*concat_in, *concat_zeros)
        jax.block_until_ready(outs)
        return [
            {n: np.asarray(outs[i]).reshape(n_cores, *out_avals[i].shape)[c]
             for i, n in enumerate(out_names)}
            for c in range(n_cores)
        ]

    return run


_CACHE = {}


def kernel(Z, weights, Q, K, V):
    """Full inputs in, full output (scalar f32 loss) out."""
    if "run" not in _CACHE:
        nc = _build_kernel(n_cores=N_CORES, reps=1)
        _CACHE["run"] = _make_runner(nc, N_CORES)
    run = _CACHE["run"]
    in_maps = [_prep_core_inputs(Z, weights, Q, K, V, c) for c in range(N_CORES)]
    res = run(in_maps)
    parts = [res[c]["partial"] for c in range(N_CORES)]
    tot = sum(-p[0, 0] for p in parts)
    return np.float32(tot + LAMBD * parts[0][0, 1])


# revision 85
# speedup vs baseline: 1.0498x; 1.0498x over previous
"""Trainium2 Bass kernel for nn_AttentionModel_PCA (sparse_attention).

loss = pseudo-likelihood of a Potts-style attention model + regularizer.

M-sharded data-parallel across 8 NeuronCores (Q/K/V replicated; scalar
partials summed on host). Per core (ML=256 of M=2048):

P1: eT_h = K_h^T Q_h on the PE in f32r; unnormalized exp(e-20) -> bf16 on
    ACT; per-(h,i) column sums via 1/128-ones matmuls; bf16 reciprocal
    row; PE-broadcast of the reciprocal; DVE fused normalize+quantize to
    sf8 = fp8_e4m3(128*softmax), layout [j_p, (jb, h, i)] so the jb pair
    feeds DoubleRow matmuls. One-hot masks O[j_p, (jb, s, m)] in fp8 via
    Pool/DVE compares (exact 0/1).
P2: Y'(h,s)[i,m] = sf8_h @ O_s as fp8 DoubleRow matmuls, s-PAIRED so each
    matmul contracts j=256 and emits [128, (s2, m)] = one full PSUM bank
    (352 matmuls total, 4-deep PSUM pipeline); evacuated with scale 2^-3
    to fp8 staging (ACT:DVE 5:9-weighted split, permuted APs reorder
    (ib,s,m)->(s,ib,m)); one DMA per h into ydA/ydB/ydC[hs, (p, ib, m)]
    with 512B-contiguous runs — the DMA does the (i,m)->partition flatten,
    and the 3-way hs-chunk split lets P3 reads start before P2 finishes.
P3: me[im, q] = sum_hs Yt[hs, im] Vcat[hs, q]: Yt superblock slabs
    prefetched via Pool-issued DMAs (SP holds its sequencer on dep waits),
    3 plain fp8 matmuls per 128-im block (DoubleRow is silently wrong on
    HW at 21-col moving width), accumulating [128, 16*21] PSUM groups;
    exp (ACT, scale 2^-4) -> DVE segment-reduce -> lsum; selected-state
    exp(me) picked by a host-built one-hot fp8 mask (DVE mult+reduce;
    tensor_tensor_reduce desyncs HW) -> esel; one Ln pass over each at
    the end: sum w*(ln esel - ln lsum) (2 act-table loads total).
Gram regularizer: 512 fp8 matmuls of sf8 columns, scaled 2^-14.

Outputs per core: [sum_m w*(sel-lge), reg']; host combines.
"""
import numpy as np
import ml_dtypes
from contextlib import ExitStack

import concourse.bass as bass
import concourse.tile as tile
from concourse import bacc, mybir

F32 = mybir.dt.float32
F32R = mybir.dt.float32r
BF16 = mybir.dt.bfloat16
I32 = mybir.dt.int32
F8 = mybir.dt.float8e4

H, D, N, S = 16, 64, 256, 21
ML = 256
HS = H * S            # 336
NB = 512              # im blocks of 128 per core (N*ML/128)
NSUP = 8              # superblocks
SUPW = 8192           # im columns per superblock
GSUP = SUPW // (16 * 128)  # me-groups per superblock
LAMBD = 0.001
N_CORES = 8
EXP_SHIFT = 20.0
SFSC = 128.0          # sf8 = 128*softmax (fp8e4m3 max is 240, so not 256)
YSC = 2.0 ** -3       # evac scale: psum holds 128*Y, store 16*Y in fp8
MESC = 2.0 ** -4      # me_psum holds 16*me
DR = mybir.MatmulPerfMode.DoubleRow
AF = mybir.ActivationFunctionType
ALU = mybir.AluOpType
AXX = mybir.AxisListType.X


def _build_kernel(n_cores=N_CORES, reps=1, dbg=False):
    nc = bacc.Bacc("TRN2", target_bir_lowering=False, debug=False,
                   num_devices=n_cores)

    k_in = nc.dram_tensor("k_in", [D, H * N], F32R, kind="ExternalInput")
    q_in = nc.dram_tensor("q_in", [D, H * N], F32R, kind="ExternalInput")
    z_in = nc.dram_tensor("z_in", [2, 128, ML], I32, kind="ExternalInput")
    wsel_in = nc.dram_tensor("wsel_in", [128, NB * S], F8, kind="ExternalInput")
    wexp_in = nc.dram_tensor("wexp_in", [128, NB], F32, kind="ExternalInput")
    vdr_in = nc.dram_tensor("vdr_in", [128, 2 * S], F8, kind="ExternalInput")
    v2_in = nc.dram_tensor("v2_in", [80, S], F8, kind="ExternalInput")
    w2_in = nc.dram_tensor("w2_in", [16, 16], F32, kind="ExternalInput")
    out_partial = nc.dram_tensor("partial", [1, 2], F32, kind="ExternalOutput")
    # split by hs-chunk so phase-3 reads only wait on the writes they need
    ydA = nc.dram_tensor("ydA", [128, 128 * 2 * ML], F8, kind="Internal")
    ydB = nc.dram_tensor("ydB", [128, 128 * 2 * ML], F8, kind="Internal")
    ydC = nc.dram_tensor("ydC", [80, 128 * 2 * ML], F8, kind="Internal")
    dbg_t = {}
    if dbg:
        dbg_t["sft8"] = nc.dram_tensor("dbg_sft8", [128, 2 * H * N], F8,
                                       kind="ExternalOutput")
        dbg_t["yd"] = nc.dram_tensor("dbg_yd", [128, 2048], F8,
                                     kind="ExternalOutput")
        dbg_t["lsum"] = nc.dram_tensor("dbg_lsum", [128, NB], F32,
                                       kind="ExternalOutput")
        dbg_t["selacc"] = nc.dram_tensor("dbg_selacc", [128, 33], F32,
                                         kind="ExternalOutput")
        dbg_t["yt01"] = nc.dram_tensor("dbg_yt01", [128, 2 * SUPW], F8,
                                       kind="ExternalOutput")
        dbg_t["yt2"] = nc.dram_tensor("dbg_yt2", [80, SUPW], F8,
                                      kind="ExternalOutput")
        dbg_t["me0"] = nc.dram_tensor("dbg_me0", [128, 16 * S], F32,
                                      kind="ExternalOutput")

    with ExitStack() as ctx:
        tc = ctx.enter_context(tile.TileContext(nc))
        pers = ctx.enter_context(tc.tile_pool(name="pers", bufs=1))
        p3y = ctx.enter_context(tc.tile_pool(name="p3y", bufs=4))
        ones_inv = pers.tile([128, 1], BF16)
        nc.vector.memset(ones_inv[:], 1.0 / SFSC)
        ones1w = pers.tile([1, 128], BF16)
        nc.vector.memset(ones1w[:], 1.0)
        ones128 = pers.tile([128, 1], F32)
        nc.vector.memset(ones128[:], 1.0)
        negshift = pers.tile([128, 1], F32)
        nc.vector.memset(negshift[:], -EXP_SHIFT)
        sft8 = pers.tile([128, 2 * H * N], F8, name="sft8")
        o8 = pers.tile([128, S * 2 * ML], F8, name="o8")
        reg_sb = pers.tile([1, 2], F32)
        w2_sb = pers.tile([16, 16], F32)
        vdr_sb = pers.tile([128, 2 * S], F8)
        v2_sb = pers.tile([80, S], F8)
        wsel_sb = pers.tile([128, NB * S], F8, name="wsel")
        wexp_sb = pers.tile([128, NB], F32)

        def _late_inputs():
            # non-P1-critical inputs: issue after P1's k/q/z DMAs
            nc.sync.dma_start(vdr_sb[:], vdr_in[:, :])
            nc.sync.dma_start(v2_sb[:], v2_in[:, :])
            nc.sync.dma_start(w2_sb[:], w2_in[:, :])
            nc.sync.dma_start(wsel_sb[:], wsel_in[:, :])
            nc.sync.dma_start(wexp_sb[:], wexp_in[:, :])

        yd = (ydA, ydB, ydC)
        for _rep in range(reps):
            _phase1(nc, tc, k_in, q_in, z_in, ones_inv, ones1w, negshift,
                    sft8, o8, _late_inputs)
            _phase2(nc, tc, sft8, o8, yd)
            _gram(nc, tc, sft8, w2_sb, ones128, reg_sb)
            if dbg:
                nc.sync.dma_start(dbg_t["sft8"][:, :], sft8[:])
                nc.sync.dma_start(dbg_t["yd"][:, :], ydA[0:128, 0:2048])
            _phase3(nc, tc, yd, vdr_sb, v2_sb, wsel_sb, wexp_sb,
                    ones128, reg_sb, out_partial, p3y, dbg_t)

    nc.compile()
    return nc


def _phase1(nc, tc, k_in, q_in, z_in, ones_inv, ones1w, negshift, sft8, o8,
            late_inputs=None):
    sft8v = sft8[:].rearrange("p (c h i) -> p c h i", c=2, h=H)
    with tc.tile_pool(name="p1", bufs=1) as p1, \
         tc.tile_pool(name="p1et", bufs=4, space="PSUM") as p1et, \
         tc.tile_pool(name="p1cs", bufs=2, space="PSUM") as p1cs, \
         tc.tile_pool(name="p1cr", bufs=2, space="PSUM") as p1cr:
        k_sb = p1.tile([D, H * N], F32R)
        nc.sync.dma_start(k_sb[:], k_in[:, :])
        q_sb = p1.tile([D, H * N], F32R)
        nc.sync.dma_start(q_sb[:], q_in[:, :])

        # one-hot masks in fp8 (exact 0/1), layout [j_p, (jb, m)] per s
        zj = p1.tile([128, 2 * ML], I32)
        nc.sync.dma_start(zj[:].rearrange("p (c m) -> p c m", c=2),
                          z_in[:, :, :].rearrange("c p m -> p c m"))
        if late_inputs is not None:
            late_inputs()
        # o8 layout [p, (jb, s, m)] so an s-pair slice gives (2, 2*ML) free
        o8v = o8[:].rearrange("p (c s m) -> p c s m", c=2, s=S)
        zjf = p1.tile([128, 2 * ML], F32)
        nc.gpsimd.tensor_copy(zjf[:], zj[:])
        zjfv = zjf[:].rearrange("p (c m) -> p c m", c=2)
        for s in range(S):
            eng = nc.gpsimd if s % 2 == 0 else nc.vector
            eng.tensor_scalar(
                out=o8v[:, :, s, :], in0=zjfv[:, :, :],
                scalar1=float(s), scalar2=None, op0=ALU.is_equal)

        sftu = p1.tile([128, 2 * H * N], BF16, name="sftu")
        sftuv = sftu[:].rearrange("p (c h i) -> p c h i", c=2, h=H)
        for h in range(H):
            cs_ps = p1cs.tile([1, N], F32, tag="cs")
            for jb in range(2):
                et_ps = p1et.tile([128, N], F32, tag="et")
                nc.tensor.matmul(
                    et_ps[:],
                    k_sb[:, h * N + jb * 128: h * N + jb * 128 + 128],
                    q_sb[:, h * N:(h + 1) * N],
                    start=True, stop=True)
                nc.scalar.activation(sftuv[:, jb, h, :], et_ps[:],
                                     AF.Exp, bias=negshift[:, :])
                nc.tensor.matmul(cs_ps[:], ones_inv[:],
                                 sftuv[:, jb, h, :],
                                 start=(jb == 0), stop=(jb == 1))
            crow = p1.tile([1, N], BF16, tag="crow")
            with nc.allow_low_precision("softmax reciprocal in bf16 is fine"):
                nc.vector.reciprocal(crow[:], cs_ps[:])
            crep_ps = p1cr.tile([128, N], F32, tag="crep")
            nc.tensor.matmul(crep_ps[:], ones1w[:], crow[:],
                             start=True, stop=True)
            for jb in range(2):
                nc.vector.tensor_tensor(
                    out=sft8v[:, jb, h, :], in0=sftuv[:, jb, h, :],
                    in1=crep_ps[:], op=ALU.mult)


def _phase2(nc, tc, sft8, o8, yd):
    sft8v = sft8[:].rearrange("p (c h i) -> p c h i", c=2, h=H)
    o8v = o8[:].rearrange("p (c s m) -> p c s m", c=2, s=S)
    bases = ((0, 0, 128), (1, 128, 128), (2, 256, 80))
    with tc.tile_pool(name="p2sb", bufs=3) as p2sb, \
         tc.tile_pool(name="p2ps", bufs=4, space="PSUM") as p2ps:
        engines = [nc.scalar, nc.vector]
        ei = 0
        for h in range(H):
            stg = p2sb.tile([128, S * 512], F8, tag="stg", name="stg")
            stgv = stg[:].rearrange("p (s i m) -> p s i m", s=S, i=2)
            for st in range(11):
                s0 = st * 2
                ns = 1 if st == 10 else 2
                # psum [p, (ib, s2, m)]: one DR matmul per ib covers both s
                ps = p2ps.tile([128, 2 * 512], F32, tag="yps")
                psv = ps[:].rearrange("p (i s m) -> p i s m", i=2, s=2)
                for ib in range(2):
                    nc.tensor.matmul(
                        psv[:, ib, 0:ns, :].rearrange("p s m -> p (s m)"),
                        sft8v[:, :, h, ib * 128:(ib + 1) * 128],
                        o8v[:, :, s0:s0 + ns, :]
                            .rearrange("p c s m -> p c (s m)"),
                        start=True, stop=True, perf_mode=DR)
                eng = engines[0 if (ei % 9) in (0, 2, 4, 6, 8) else 1]
                ei += 1
                # evac psum (ib, s2, m) -> staging (s, ib, m): permuted APs
                dst = stgv[:, s0:s0 + ns, :, :]
                src = psv[:, :, 0:ns, :].rearrange("p i s m -> p s i m")
                if eng is nc.scalar:
                    nc.scalar.mul(dst, src, YSC)
                else:
                    eng.tensor_scalar(out=dst, in0=src, scalar1=YSC,
                                      scalar2=None, op0=ALU.mult)
            lo, hi = h * S, (h + 1) * S
            for t_idx, base, size in bases:
                a, b = max(lo, base), min(hi, base + size)
                if a >= b:
                    continue
                s0, s1 = a - lo, b - lo
                nc.sync.dma_start(
                    yd[t_idx][a - base:b - base, :]
                        .rearrange("s (p x) -> p s x", p=128),
                    stg[:, s0 * 512:s1 * 512]
                        .rearrange("p (s x) -> p s x", s=s1 - s0))


def _gram(nc, tc, sft8, w2_sb, ones128, reg_sb):
    sft8g = sft8[:].rearrange("p (c h i) -> p c h i", c=2, h=H)
    with tc.tile_pool(name="gr", bufs=1) as gr, \
         tc.tile_pool(name="grps", bufs=1, space="PSUM") as grps:
        mm_ps = grps.tile([16, 16], F32, tag="mm")
        for i in range(N):
            for jb in range(2):
                nc.tensor.matmul(mm_ps[:], sft8g[:, jb, :, i],
                                 sft8g[:, jb, :, i],
                                 start=(i == 0 and jb == 0),
                                 stop=(i == N - 1 and jb == 1))
        mw = gr.tile([16, 16], F32)
        nc.vector.tensor_tensor(out=mw[:], in0=mm_ps[:], in1=w2_sb[:],
                                op=ALU.mult)
        mwr = gr.tile([16, 1], F32)
        nc.vector.reduce_sum(mwr[:], mw[:], axis=AXX)
        reg_ps = grps.tile([1, 1], F32, tag="regp")
        nc.tensor.matmul(reg_ps[:], mwr[:], ones128[:16, :], start=True,
                         stop=True)
        nc.vector.tensor_copy(reg_sb[:, 1:2], reg_ps[:])


def _phase3(nc, tc, yd, vdr_sb, v2_sb, wsel_sb, wexp_sb, ones128,
            reg_sb, out_partial, p3y, dbg_t=None):
    vdrv = vdr_sb[:].rearrange("p (c q) -> p c q", c=2)
    with tc.tile_pool(name="p3", bufs=1) as p3, \
         tc.tile_pool(name="p3x", bufs=3) as p3x, \
         tc.tile_pool(name="p3ps", bufs=4, space="PSUM") as p3ps:
        lse2 = p3.tile([128, 2 * NB], F32, name="lse2")
        lse2v = lse2[:].rearrange("p (t b) -> p t b", t=2)
        lsum = lse2v[:, 0, :]
        esel = lse2v[:, 1, :]
        for sup in range(NSUP):
            yt01 = p3y.tile([128, 2 * SUPW], F8, tag="yt01", name="yt01")
            yt01v_ = yt01[:].rearrange("p (c x) -> p c x", c=2)
            nc.gpsimd.dma_start(yt01v_[:, 0, :],
                                yd[0][:, sup * SUPW:(sup + 1) * SUPW])
            nc.gpsimd.dma_start(yt01v_[:, 1, :],
                                yd[1][:, sup * SUPW:(sup + 1) * SUPW])
            yt2 = p3y.tile([80, SUPW], F8, tag="yt2", name="yt2")
            nc.gpsimd.dma_start(yt2[:], yd[2][:, sup * SUPW:(sup + 1) * SUPW])
            yt01v = yt01[:].rearrange("p (c x) -> p c x", c=2)
            for g in range(GSUP):
                me = p3ps.tile([128, 16 * S], F32, tag="me")
                for blk in range(16):
                    off = (g * 16 + blk) * 128
                    nc.tensor.matmul(me[:, blk * S:(blk + 1) * S],
                                     yt01v[:, 0, off:off + 128], vdrv[:, 0, :],
                                     start=True, stop=False)
                    nc.tensor.matmul(me[:, blk * S:(blk + 1) * S],
                                     yt01v[:, 1, off:off + 128], vdrv[:, 1, :],
                                     start=False, stop=False)
                    nc.tensor.matmul(me[:, blk * S:(blk + 1) * S],
                                     yt2[:, off:off + 128], v2_sb[:, :],
                                     start=False, stop=True)
                gg = sup * GSUP + g
                if dbg_t and sup == 0 and g == 0:
                    me_dbg = p3x.tile([128, 16 * S], F32, tag="medbg")
                    nc.vector.tensor_copy(me_dbg[:], me[:])
                    nc.sync.dma_start(dbg_t["me0"][:, :], me_dbg[:])
                    nc.sync.dma_start(dbg_t["yt01"][:, :], yt01[:])
                    nc.sync.dma_start(dbg_t["yt2"][:, :], yt2[:])
                expo = p3x.tile([128, 16 * S], F32, tag="expo")
                nc.scalar.activation(expo[:], me[:], AF.Exp, scale=MESC)
                nc.vector.reduce_sum(
                    lsum[:, gg * 16:(gg + 1) * 16],
                    expo[:].rearrange("p (b q) -> p b q", q=S), axis=AXX)
                scr = p3x.tile([128, 16 * S], F32, tag="scr")
                nc.vector.tensor_tensor(
                    out=scr[:], in0=expo[:],
                    in1=wsel_sb[:, gg * 16 * S:(gg + 1) * 16 * S],
                    op=ALU.mult)
                nc.vector.reduce_sum(
                    esel[:, gg * 16:(gg + 1) * 16],
                    scr[:].rearrange("p (b q) -> p b q", q=S), axis=AXX)
        if dbg_t:
            nc.sync.dma_start(dbg_t["lsum"][:, :], lsum)
            nc.sync.dma_start(dbg_t["selacc"][:, 0:32], lse2[:, NB:NB + 32])
        lge = p3.tile([128, NB], F32)
        nc.scalar.activation(lge[:], lsum, AF.Ln)
        lsel = p3.tile([128, NB], F32)
        nc.scalar.activation(lsel[:], esel, AF.Ln)
        dif = p3.tile([128, NB], F32)
        nc.vector.tensor_sub(dif[:], lsel[:], lge[:])
        scr2 = p3.tile([128, NB], F32)
        nc.vector.tensor_tensor(out=scr2[:], in0=dif[:], in1=wexp_sb[:],
                                op=ALU.mult)
        diff = p3.tile([128, 1], F32)
        nc.vector.reduce_sum(diff[:], scr2[:], axis=AXX)
        with tc.tile_pool(name="plps", bufs=1, space="PSUM") as plps:
            pl_ps = plps.tile([1, 1], F32, tag="pl")
            nc.tensor.matmul(pl_ps[:], diff[:], ones128[:], start=True,
                             stop=True)
            nc.vector.tensor_copy(reg_sb[:, 0:1], pl_ps[:])
        nc.sync.dma_start(out_partial[:, :], reg_sb[:])


# ===================== host side: shard, run, combine =====================

F8NP = ml_dtypes.float8_e4m3


def _prep_core_inputs(Z, weights, Q, K, V, core, n_cores=N_CORES):
    Z = np.asarray(Z)
    w = np.asarray(weights, np.float32)
    msl = slice(core * ML, (core + 1) * ML)
    zc = np.ascontiguousarray(Z[:, msl]).astype(np.int32)     # [j=256, m=256]
    wc = w[msl]                                               # [256]

    # block b = p*4 + ib*2 + mh ; partition ml ; i = ib*128+p ; m = mh*128+ml
    b = np.arange(NB)
    p_, ib_, mh_ = b // 4, (b // 2) % 2, b % 2
    i_of_b = ib_ * 128 + p_                                   # [NB]
    ml_ = np.arange(128)
    m_of = mh_[None, :] * 128 + ml_[:, None]                  # [128, NB]
    zsel = zc[i_of_b[None, :].repeat(128, 0), m_of]           # [128, NB]
    wm = wc[m_of]                                             # [128, NB]
    q_ar = np.arange(S)
    wsel = (zsel[:, :, None] == q_ar[None, None, :]).astype(np.float32)
    wsel = np.ascontiguousarray(wsel.reshape(128, NB * S)).astype(F8NP)

    vcat = np.transpose(np.asarray(V, np.float32), (0, 2, 1)).reshape(HS, S)
    v8 = vcat.astype(F8NP)
    vdr = np.empty((128, 2, S), F8NP)
    vdr[:, 0, :] = v8[0:128]
    vdr[:, 1, :] = v8[128:256]
    v2 = np.ascontiguousarray(v8[256:336])

    vv = np.asarray(V, np.float32).reshape(H, -1)
    w2 = (vv @ vv.T) / (SFSC * SFSC)

    return {
        "k_in": np.ascontiguousarray(
            np.asarray(K, np.float32).transpose(1, 0, 2).reshape(D, H * N)),
        "q_in": np.ascontiguousarray(
            np.asarray(Q, np.float32).transpose(1, 0, 2).reshape(D, H * N)),
        "z_in": np.ascontiguousarray(zc.reshape(2, 128, ML)),
        "wsel_in": wsel,
        "wexp_in": np.ascontiguousarray(wm.astype(np.float32)),
        "vdr_in": np.ascontiguousarray(vdr.reshape(128, 2 * S)),
        "v2_in": v2,
        "w2_in": np.ascontiguousarray(w2.astype(np.float32)),
    }


def _make_runner(nc, n_cores):
    """jit once; reuse. Based on bass2jax.run_bass_via_pjrt (axon path)."""
    import jax
    from jax.sharding import Mesh, PartitionSpec
    from jax.experimental.shard_map import shard_map
    from concourse import bass2jax

    bass2jax.install_neuronx_cc_hook()
    partition_name = (nc.partition_id_tensor.name
                      if nc.partition_id_tensor else None)
    in_names, out_names, out_avals, zero_outs = [], [], [], []
    for alloc in nc.m.functions[0].allocations:
        if not isinstance(alloc, mybir.MemoryLocationSet):
            continue
        name = alloc.memorylocations[0].name
        if alloc.kind == "ExternalInput":
            if name != partition_name:
                in_names.append(name)
        elif alloc.kind == "ExternalOutput":
            out_names.append(name)
            shape = tuple(alloc.tensor_shape)
            dtype = mybir.dt.np(alloc.dtype)
            out_avals.append(jax.core.ShapedArray(shape, dtype))
            zero_outs.append(np.zeros(shape, dtype))
    n_params = len(in_names)
    n_outs = len(out_names)
    all_in_names = in_names + out_names
    if partition_name is not None:
        all_in_names = all_in_names + [partition_name]

    def _body(*args):
        operands = list(args)
        if partition_name is not None:
            operands.append(bass2jax.partition_id_tensor())
        outs = bass2jax._bass_exec_p.bind(
            *operands,
            out_avals=tuple(out_avals),
            in_names=tuple(all_in_names),
            out_names=tuple(out_names),
            lowering_input_output_aliases=(),
            sim_require_finite=True,
            sim_require_nnan=True,
            nc=nc,
        )
        return tuple(outs)

    donate = tuple(range(n_params, n_params + n_outs))
    devices = jax.devices()[:n_cores]
    mesh = Mesh(np.asarray(devices), ("core",))
    in_specs = (PartitionSpec("core"),) * (n_params + n_outs)
    out_specs = (PartitionSpec("core"),) * n_outs
    jf = jax.jit(
        shard_map(_body, mesh=mesh, in_specs=in_specs, out_specs=out_specs,
                  check_rep=False),
        donate_argnums=donate, keep_unused=True,
    )

    def run(in_maps):
        concat_in = [
            np.concatenate([np.asarray(in_maps[c][n]) for c in range(n_cores)],
                           axis=0)
            for n in in_names
        ]
        concat_zeros = [
            np.zeros((n_cores * z.shape[0], *z.shape[1:]), z.dtype)
            for z in zero_outs
        ]
        outs = jf(*concat_in, *concat_zeros)
        jax.block_until_ready(outs)
        return [
            {n: np.asarray(outs[i]).reshape(n_cores, *out_avals[i].shape)[c]
             for i, n in enumerate(out_names)}
            for c in range(n_cores)
        ]

    return run


_CACHE = {}


def kernel(Z, weights, Q, K, V):
    """Full inputs in, full output (scalar f32 loss) out."""
    if "run" not in _CACHE:
        nc = _build_kernel(n_cores=N_CORES, reps=1)
        _CACHE["run"] = _make_runner(nc, N_CORES)
    run = _CACHE["run"]
    in_maps = [_prep_core_inputs(Z, weights, Q, K, V, c) for c in range(N_CORES)]
    res = run(in_maps)
    parts = [res[c]["partial"] for c in range(N_CORES)]
    tot = sum(-p[0, 0] for p in parts)
    return np.float32(tot + LAMBD * parts[0][0, 1])
